# revision 1
# baseline (speedup 1.0000x reference)
"""Trainium2 Bass kernel for the Adapt_Layer MoE-routing problem.

Full-input interface: kernel(**inputs) -> np.ndarray [B, D] float32.
Data-parallel over 8 NeuronCores: batch B=16384 sharded 2048/core,
C=8 stacked expert weights replicated.

Math (per batch row x, probs p):
    expert_c = x @ W[c].T + b[c]
    pred     = sum_c p[c] * expert_c
             = sum_c p[c] * (x @ W[c].T)  +  p @ b          (K=8 matmul)
    s_p      = pred @ pw_w + pw_b = sum_c p[c]*(x @ v_c + beta_c) + pw_b
                  with v_c = W[c].T @ pw_w, beta_c = b[c] @ pw_w  (host-precomputed)
    s_f      = x @ fw_w + fw_b
    out      = sigmoid(s_p) * pred + sigmoid(s_f) * x

Device layout: batch rows on partitions (128/B-tile), so the prob weighting and
sigmoid gates are native per-partition-scalar ops. The big einsum runs as
bf16 matmuls with feature^T k-chunks stationary, reused across all 8 experts
(one PSUM bank each), fp32 accumulation in PSUM.
"""

import os
import sys
from contextlib import ExitStack

import numpy as np

sys.path.insert(0, "/opt/trn_rl_repo")

import ml_dtypes

import concourse.bass as bass
import concourse.mybir as mybir
import concourse.tile as tile
from concourse import bacc
from concourse.bass_utils import run_bass_kernel_spmd

BF16 = ml_dtypes.bfloat16

B, D, C = 16384, 1024, 8
NCORES = 8
BL = B // NCORES          # 2048 batch rows per core
P = 128                   # partitions
NBT = BL // P             # 16 B-tiles per core
KC = D // P               # 8 k-chunks
H = 512                   # output half width (one PSUM bank of fp32)

# Set by the last run when tracing is enabled (KERNEL_TRACE=1)
LAST_EXEC_NS = None
LAST_RESULTS = None


def _install_profile_shim():
    """Enable NTFF profiling under axon: provide the antenv.axon_hooks module
    the boot shim expects, wire the ctypes hook, and keep artifacts local."""
    import types

    import concourse.bass_utils as bu

    bu.upload_artifacts = lambda tmpdir: tmpdir
    try:
        import antenv.axon_hooks  # noqa: F401
        return
    except ImportError:
        pass
    import antenv

    mod = types.ModuleType("antenv.axon_hooks")
    _h = [None]
    mod.set_axon_ntff_profile_hook = lambda h: _h.__setitem__(0, h)
    mod.get_axon_ntff_profile_hook = lambda: _h[0]
    sys.modules["antenv.axon_hooks"] = mod
    antenv.axon_hooks = mod
    try:
        from trn_agent_boot.trn_boot import _ntff_profile_via_ctypes

        hook = _ntff_profile_via_ctypes("/opt/axon/libaxon_pjrt.so")
        if hook is not None:
            mod.set_axon_ntff_profile_hook(hook)
    except Exception as e:  # profiling is best-effort
        print(f"profile shim failed: {e}")


def _dedupe_ldweights(nc) -> int:
    """Drop InstLdweights that reload the exact weights already in the PE
    array (same weights AP as the previous Ldweights, nothing in between
    that changes the array, no semaphore traffic attached). Tile's
    legalizer emits one Ldweights per matmul; with a stationary operand
    reused across 8 experts, 7 of 8 loads are redundant."""
    dropped = 0
    for f in nc.m.functions:
        for blk in f.blocks:
            insts = blk.instructions
            keep = []
            last_sig = None
            for inst in insts:
                tn = type(inst).__name__
                if tn == "InstLdweights":
                    sig = str(inst.ins[0])
                    si = inst.sync_info
                    empty = si is None or (not si.on_wait and not si.on_update)
                    if empty and sig == last_sig:
                        dropped += 1
                        continue
                    last_sig = sig
                keep.append(inst)
            if dropped:
                blk.instructions = keep
    return dropped


def _build_graph(pw_b_f: float) -> bass.Bass:
    f32 = mybir.dt.float32
    bf16 = mybir.dt.bfloat16
    AF = mybir.ActivationFunctionType
    ALU = mybir.AluOpType

    nc = bacc.Bacc()
    featT_p = nc.declare_dram_parameter("featT", [D, BL], bf16, isOutput=False)
    feat_p = nc.declare_dram_parameter("feat", [BL, D], bf16, isOutput=False)
    w_p = nc.declare_dram_parameter("w", [C * D, D], bf16, isOutput=False)
    prob_p = nc.declare_dram_parameter("prob", [BL, C], f32, isOutput=False)
    probT_p = nc.declare_dram_parameter("probT", [C, BL], bf16, isOutput=False)
    gmat_p = nc.declare_dram_parameter("gmat", [P, KC * 9], bf16, isOutput=False)
    gbeta_p = nc.declare_dram_parameter("gbeta", [1, 9], bf16, isOutput=False)
    bb_p = nc.declare_dram_parameter("bb", [C, D], bf16, isOutput=False)
    out_p = nc.declare_dram_parameter("out", [BL, D], f32, isOutput=True)

    with ExitStack() as ctx:
        tc = ctx.enter_context(tile.TileContext(nc))

        const = ctx.enter_context(tc.tile_pool(name="const", bufs=1))
        psum = ctx.enter_context(tc.tile_pool(name="psum", bufs=1, space="PSUM"))
        prob_pool = ctx.enter_context(tc.tile_pool(name="probp", bufs=3))
        feat_pool = ctx.enter_context(tc.tile_pool(name="featp", bufs=2))
        acc_pool = ctx.enter_context(tc.tile_pool(name="accp", bufs=2))
        tmp_pool = ctx.enter_context(tc.tile_pool(name="tmpp", bufs=4))
        gate_pool = ctx.enter_context(tc.tile_pool(name="gatep", bufs=3))

        # ---- resident inputs ----
        # Issue order matters: the Sync sequencer dispatches ~0.7us per DMA,
        # so first-needed tiles go first and the 64 W transfers are issued
        # from the (otherwise idle) GpSimd sequencer in parallel.
        featT_sb = const.tile([P, KC * BL], bf16)          # [:, k*BL : k*BL+BL]
        w_sb = const.tile([P, C * KC * D], bf16)           # [:, (c*KC+k)*D : +D]
        # featT[0] + gmat first: they gate the pre-pass's first matmul
        nc.sync.dma_start(featT_sb[:, 0:BL], featT_p[0:P, :])
        gmat_sb = const.tile([P, KC * 9], bf16)
        nc.sync.dma_start(gmat_sb[:], gmat_p[:])
        probT_sb = const.tile([C, BL], bf16)
        nc.sync.dma_start(probT_sb[:], probT_p[:])
        bb_sb = const.tile([C, D], bf16)
        nc.sync.dma_start(bb_sb[:], bb_p[:])
        gbeta_sb = const.tile([1, 9], bf16)
        nc.sync.dma_start(gbeta_sb[:], gbeta_p[:])
        prob_all = const.tile([P, NBT * C], f32)           # [:, bt*C+c]
        nc.sync.dma_start(
            prob_all[:].rearrange("p (t c) -> p t c", c=C),
            prob_p.rearrange("(t p) c -> p t c", p=P),
        )
        # rest of featT as one batched transfer (chunks are contiguous in
        # both DRAM and SBUF; the gate pre-pass paces through it while W
        # streams)
        nc.sync.dma_start(
            featT_sb[:, BL : KC * BL].rearrange("p (k b) -> p k b", b=BL),
            featT_p[P : KC * P, :].rearrange("(k p) b -> p k b", p=P),
        )
        # W k-major (first B-tile consumes k=0 for all c first). Consecutive
        # k-chunks are contiguous in both DRAM and SBUF, so move two per DMA
        # — halves the ~0.7us-per-DMA Sync issue traffic.
        for k2 in range(0, KC, 2):
            for c in range(C):
                nc.sync.dma_start(
                    w_sb[:, (c * KC + k2) * D : (c * KC + k2 + 2) * D].rearrange(
                        "p (k d) -> p k d", k=2
                    ),
                    w_p[c * D + k2 * P : c * D + (k2 + 2) * P, :].rearrange(
                        "(k p) d -> p k d", p=P
                    ),
                )
        ones1 = const.tile([1, P], bf16)
        nc.vector.memset(ones1[:], 1.0)
        pwb_sb = const.tile([P, 1], f32)
        nc.vector.memset(pwb_sb[:], pw_b_f)
        zero_sb = const.tile([P, 1], f32)
        nc.vector.memset(zero_sb[:], 0.0)

        def lhs_feat(k, bt):
            return featT_sb[:, k * BL + bt * P : k * BL + (bt + 1) * P]

        # ---- gate pre-pass: all 16 B-tiles' gate logits, computed while the
        # 16MB W stream saturates HBM (the PE would otherwise stall). Only
        # needs featT (4MB) + gmat. Uses banks e0..e5 so bt0's bias matmuls
        # (e6/e7) and early expert work are not blocked behind it.
        pred_w_all = const.tile([P, NBT], f32)
        fw_all = const.tile([P, NBT], f32)
        for chunk0 in range(0, NBT, 6):
            bts = range(chunk0, min(chunk0 + 6, NBT))
            pgs = {
                bt: psum.tile([P, 9], f32, tag=f"e{bt % 6}", name="pg") for bt in bts
            }
            # k-outer so the chunk's gate matmuls start as soon as featT[k]
            # arrives, pacing the PE through the DMA-bound startup window
            for k in range(KC):
                for bt in bts:
                    nc.tensor.matmul(
                        pgs[bt][:],
                        lhs_feat(k, bt),
                        gmat_sb[:, k * 9 : (k + 1) * 9],
                        start=(k == 0),
                        stop=False,
                    )
            for bt in bts:
                nc.tensor.matmul(
                    pgs[bt][:], ones1[:], gbeta_sb[:], start=False, stop=True
                )
            for bt in bts:
                sg = gate_pool.tile([P, 9], f32, name="sg")
                nc.vector.tensor_copy(sg[:], pgs[bt][:])
                junk = gate_pool.tile([P, C], f32, name="junk")
                junk2 = gate_pool.tile([P, C], f32, name="junk2")
                sp = gate_pool.tile([P, 1], f32, name="sp")
                # (tensor_tensor_reduce faults the DVE on this HW — use
                # DVE multiply + ACT free-dim accumulator instead)
                nc.vector.tensor_tensor(
                    junk[:], sg[:, 0:C], prob_all[:, bt * C : (bt + 1) * C], op=ALU.mult
                )
                nc.scalar.activation(junk2[:], junk[:], AF.Copy, accum_out=sp[:])
                nc.scalar.activation(
                    pred_w_all[:, bt : bt + 1], sp[:], AF.Sigmoid, bias=pwb_sb[:]
                )
                nc.scalar.activation(
                    fw_all[:, bt : bt + 1], sg[:, C : C + 1], AF.Sigmoid, bias=zero_sb[:]
                )

        for bt in range(NBT):
            acc = acc_pool.tile([P, D], f32, bufs=3)

            # ---- prob-weighted bias (no featT dependency — runs at start) ----
            # lives in the banks of experts 6/7, which the next half's k-loop
            # reaches last (most slack for the WAR on the bias evacuation)
            pb0 = psum.tile([P, H], f32, tag="e6")
            pb1 = psum.tile([P, H], f32, tag="e7")
            lhs_probT = probT_sb[:, bt * P : (bt + 1) * P]
            nc.tensor.matmul(pb0[:], lhs_probT, bb_sb[:, 0:H], start=True, stop=True)
            nc.tensor.matmul(pb1[:], lhs_probT, bb_sb[:, H:D], start=True, stop=True)
            nc.vector.tensor_copy(acc[:, 0:H], pb0[:])
            nc.vector.tensor_copy(acc[:, H:D], pb1[:])

            # ---- main expert matmuls: 2 halves x 8 k-chunks x 8 experts ----
            for h in range(2):
                pe = [psum.tile([P, H], f32, tag=f"e{c}", name=f"pe{c}") for c in range(C)]
                # h0 walks k up (ends at k=7), h1 walks k down (starts at
                # k=7) so the half-boundary reuses the loaded stationary
                korder = list(range(KC)) if h == 0 else list(range(KC - 1, -1, -1))
                for ki, k in enumerate(korder):
                    lhs = lhs_feat(k, bt)
                    for c in range(C):
                        nc.tensor.matmul(
                            pe[c][:],
                            lhs,
                            w_sb[:, (c * KC + k) * D + h * H : (c * KC + k) * D + h * H + H],
                            start=(ki == 0),
                            stop=(ki == KC - 1),
                        )
                acch = acc[:, h * H : (h + 1) * H]
                # ACT is dedicated to these bank-freeing copies (they gate
                # PE's psum reuse); all other elementwise work lives on DVE
                for c in range(C):
                    t = tmp_pool.tile([P, H], f32, name=f"t{c}", tag="t", bufs=8)
                    nc.scalar.activation(
                        t[:], pe[c][:], AF.Copy,
                        scale=prob_all[:, bt * C + c : bt * C + c + 1],
                    )
                    nc.vector.tensor_tensor(acch, acch, t[:], op=ALU.add)

            # epilogue-only input; issued late so the early HBM bandwidth
            # goes to the W stream
            feat_sb = feat_pool.tile([P, D], bf16)
            nc.sync.dma_start(feat_sb[:], feat_p[bt * P : (bt + 1) * P, :])

            # ---- epilogue: out = sigmoid(s_p)*pred + sigmoid(s_f)*feature ----
            # per half: ACT does the pred_w scaling in parallel with DVE's
            # fw*feature, then one DVE add and the half stores immediately
            for h in range(2):
                acch = acc[:, h * H : (h + 1) * H]
                nc.scalar.activation(
                    acch, acch, AF.Copy, scale=pred_w_all[:, bt : bt + 1]
                )
                ft = tmp_pool.tile([P, H], f32, tag="ft", bufs=2)
                nc.vector.tensor_scalar_mul(
                    ft[:], feat_sb[:, h * H : (h + 1) * H], fw_all[:, bt : bt + 1]
                )
                nc.vector.tensor_tensor(acch, acch, ft[:], op=ALU.add)
                nc.sync.dma_start(
                    out_p[bt * P : (bt + 1) * P, h * H : (h + 1) * H], acch
                )

    if os.environ.get("KERNEL_NO_LDW_DEDUPE") != "1":
        _dedupe_ldweights(nc)
    nc.compile()
    return nc


def kernel(feature, prob, W, b, pw_w, pw_b, fw_w, fw_b):
    global LAST_EXEC_NS, LAST_RESULTS
    feature = np.asarray(feature, dtype=np.float32)
    prob = np.asarray(prob, dtype=np.float32)
    W = np.asarray(W, dtype=np.float32)
    b = np.asarray(b, dtype=np.float32)
    pw_w = np.asarray(pw_w, dtype=np.float32)
    fw_w = np.asarray(fw_w, dtype=np.float32)
    pw_b_f = float(np.asarray(pw_b).reshape(-1)[0])
    fw_b_f = float(np.asarray(fw_b).reshape(-1)[0])

    # host-side weight prep (replicated across cores)
    w_host = np.ascontiguousarray(W.transpose(0, 2, 1)).reshape(C * D, D).astype(BF16)
    G = np.einsum("cod,o->dc", W, pw_w)                      # [D, C]: v_c columns
    G9 = np.concatenate([G, fw_w[:, None]], axis=1)          # [D, 9]
    gmat = np.ascontiguousarray(
        G9.reshape(KC, P, 9).transpose(1, 0, 2).reshape(P, KC * 9)
    ).astype(BF16)
    gbeta = np.concatenate([b @ pw_w, [fw_b_f]]).reshape(1, 9).astype(BF16)
    bb = b.astype(BF16)

    in_maps = []
    for i in range(NCORES):
        sl = slice(i * BL, (i + 1) * BL)
        in_maps.append(
            {
                "featT": np.ascontiguousarray(feature[sl].T).astype(BF16),
                "feat": feature[sl].astype(BF16),
                "w": w_host,
                "prob": np.ascontiguousarray(prob[sl]),
                "probT": np.ascontiguousarray(prob[sl].T).astype(BF16),
                "gmat": gmat,
                "gbeta": gbeta,
                "bb": bb,
            }
        )

    nc = _build_graph(pw_b_f)
    trace = bool(int(os.environ.get("KERNEL_TRACE", "0")))
    if trace:
        _install_profile_shim()
    res = run_bass_kernel_spmd(
        nc, in_maps, core_ids=list(range(NCORES)), trace=trace
    )
    LAST_EXEC_NS = res.exec_time_ns
    LAST_RESULTS = res
    out = np.concatenate([res.results[i]["out"] for i in range(NCORES)], axis=0)
    return np.asarray(out, dtype=np.float32)



# revision 8
# speedup vs baseline: 1.3333x; 1.3333x over previous
"""Trainium2 Bass kernel for the Adapt_Layer MoE-routing problem.

Full-input interface: kernel(**inputs) -> np.ndarray [B, D] float32.
Data-parallel over 8 NeuronCores: batch B=16384 sharded 2048/core,
C=8 stacked expert weights replicated.

Math (per batch row x, probs p):
    expert_c = x @ W[c].T + b[c]
    pred     = sum_c p[c] * expert_c = sum_c (p[c]*x) @ W[c].T + p @ b
    s_p      = pred @ pw_w + pw_b    (gate pre-pass via v_c = W[c].T@pw_w)
    s_f      = x @ fw_w + fw_b
    out      = sigmoid(s_p) * pred + sigmoid(s_f) * x

Device dataflow (the key restructure vs a per-expert-PSUM design):
the prob weighting is folded into the STATIONARY operand: featT_sc[c] =
featT * p[b, c] (built on DVE from a GpSimd-replicated p tile). All 8
experts AND the p@b bias then accumulate into a single PSUM bank per
output half, so evacuation is one ACT per (b-tile, half) instead of 8
copy+add pairs. Optionally the first 2*NPAIR k-chunks of the
contraction run as fp8-e4m3 DoubleRow matmuls (2 k-chunks per
instruction) into a separate bank pair, merged at evacuation with a
1/128 scale (W8 = e4m3(128*W)).
"""

import os
import sys
from contextlib import ExitStack

import numpy as np

sys.path.insert(0, "/opt/trn_rl_repo")

import ml_dtypes

import concourse.bass as bass
import concourse.mybir as mybir
import concourse.tile as tile
from concourse import bacc
from concourse.bass_utils import run_bass_kernel_spmd

BF16 = ml_dtypes.bfloat16
FP8 = ml_dtypes.float8_e4m3fn

B, D, C = 16384, 1024, 8
NCORES = 8
P = 128                   # partitions
NBT = 16                  # B-tiles per core
BL = NBT * P              # batch rows per core
KC = D // P               # 8 k-chunks
H = 512                   # output half width (one PSUM bank of fp32)
NPAIR = int(os.environ.get("KERNEL_NPAIR", "2"))  # fp8 DoubleRow k-pairs (0..4)
KB = KC - 2 * NPAIR       # bf16 k-chunks
KOFF = 2 * NPAIR          # first bf16 k-chunk
W8SCALE = 128.0           # host scale on W8; merged back as 1/128 at evac

# Set by the last run when tracing is enabled (KERNEL_TRACE=1)
LAST_EXEC_NS = None
LAST_RESULTS = None


def _install_profile_shim():
    """Enable NTFF profiling under axon: provide the antenv.axon_hooks module
    the boot shim expects, wire the ctypes hook, and keep artifacts local."""
    import types

    import concourse.bass_utils as bu

    bu.upload_artifacts = lambda tmpdir: tmpdir
    try:
        import antenv.axon_hooks  # noqa: F401
        return
    except ImportError:
        pass
    import antenv

    mod = types.ModuleType("antenv.axon_hooks")
    _h = [None]
    mod.set_axon_ntff_profile_hook = lambda h: _h.__setitem__(0, h)
    mod.get_axon_ntff_profile_hook = lambda: _h[0]
    sys.modules["antenv.axon_hooks"] = mod
    antenv.axon_hooks = mod
    try:
        from trn_agent_boot.trn_boot import _ntff_profile_via_ctypes

        hook = _ntff_profile_via_ctypes("/opt/axon/libaxon_pjrt.so")
        if hook is not None:
            mod.set_axon_ntff_profile_hook(hook)
    except Exception as e:  # profiling is best-effort
        print(f"profile shim failed: {e}")


def _dedupe_ldweights(nc) -> int:
    """Drop InstLdweights that reload the exact weights already in the PE
    array (same weights AP as the previous Ldweights, nothing attached)."""
    dropped = 0
    for f in nc.m.functions:
        for blk in f.blocks:
            insts = blk.instructions
            keep = []
            last_sig = None
            for inst in insts:
                tn = type(inst).__name__
                if tn == "InstLdweights":
                    sig = str(inst.ins[0]) + str(getattr(inst, "perf_mode", None))
                    si = inst.sync_info
                    empty = si is None or (not si.on_wait and not si.on_update)
                    if empty and sig == last_sig:
                        dropped += 1
                        continue
                    last_sig = sig
                keep.append(inst)
            if dropped:
                blk.instructions = keep
    return dropped


def _build_graph(pw_b_f: float) -> bass.Bass:
    f32 = mybir.dt.float32
    bf16 = mybir.dt.bfloat16
    fp8e4 = mybir.dt.float8e4
    AF = mybir.ActivationFunctionType
    ALU = mybir.AluOpType
    DR = mybir.MatmulPerfMode.DoubleRow

    nc = bacc.Bacc()
    featT_p = nc.declare_dram_parameter("featT", [D, BL], bf16, isOutput=False)
    feat_p = nc.declare_dram_parameter("feat", [BL, D], bf16, isOutput=False)
    if KB:
        wb_p = nc.declare_dram_parameter("wb", [C * KB * P, D], bf16, isOutput=False)
    if NPAIR:
        w8_p = nc.declare_dram_parameter(
            "w8", [C * NPAIR * P, 2 * D], fp8e4, isOutput=False
        )
    prob_p = nc.declare_dram_parameter("prob", [BL, C], f32, isOutput=False)
    probT_p = nc.declare_dram_parameter("probT", [C, BL], bf16, isOutput=False)
    probT32_p = nc.declare_dram_parameter("probT32", [C, BL], f32, isOutput=False)
    gmat_p = nc.declare_dram_parameter("gmat", [P, KC * 9], bf16, isOutput=False)
    gbeta_p = nc.declare_dram_parameter("gbeta", [1, 9], bf16, isOutput=False)
    bb_p = nc.declare_dram_parameter("bb", [C, D], bf16, isOutput=False)
    out_p = nc.declare_dram_parameter("out", [BL, D], f32, isOutput=True)

    with ExitStack() as ctx:
        tc = ctx.enter_context(tile.TileContext(nc))

        const = ctx.enter_context(tc.tile_pool(name="const", bufs=1))
        psum = ctx.enter_context(tc.tile_pool(name="psum", bufs=1, space="PSUM"))
        prep_pool = ctx.enter_context(tc.tile_pool(name="prepp", bufs=1))
        sc_pool = ctx.enter_context(tc.tile_pool(name="scp", bufs=1))
        feat_pool = ctx.enter_context(tc.tile_pool(name="featp", bufs=2))
        stage_pool = ctx.enter_context(tc.tile_pool(name="stgp", bufs=1))
        gate_pool = ctx.enter_context(tc.tile_pool(name="gatep", bufs=3))

        # ---- resident inputs ----
        # Issue order = consumption order: featT feeds the gate pre-pass and
        # the prescale DVE; w8 feeds the first (fp8) matmul groups; wb is
        # consumed last within each b-tile.
        featT_sb = const.tile([P, KC * BL], bf16)          # [:, k*BL : k*BL+BL]
        nc.sync.dma_start(featT_sb[:, 0:BL], featT_p[0:P, :])
        gmat_sb = const.tile([P, KC * 9], bf16)
        nc.sync.dma_start(gmat_sb[:], gmat_p[:])
        probT_sb = const.tile([C, BL], bf16)
        nc.sync.dma_start(probT_sb[:], probT_p[:])
        bb_sb = const.tile([C, D], bf16)
        nc.sync.dma_start(bb_sb[:], bb_p[:])
        gbeta_sb = const.tile([1, 9], bf16)
        nc.sync.dma_start(gbeta_sb[:], gbeta_p[:])
        prob_all = const.tile([P, NBT * C], f32)           # [:, bt*C+c]
        nc.sync.dma_start(
            prob_all[:].rearrange("p (t c) -> p t c", c=C),
            prob_p.rearrange("(t p) c -> p t c", p=P),
        )
        # rest of featT (prepass + prescale pace through it)
        nc.sync.dma_start(
            featT_sb[:, BL : KC * BL].rearrange("p (k b) -> p k b", b=BL),
            featT_p[P : KC * P, :].rearrange("(k p) b -> p k b", p=P),
        )
        if NPAIR:
            w8_sb = const.tile([P, C * NPAIR * 2 * D], fp8e4)
            for kp in range(NPAIR):
                for c in range(C):
                    i = c * NPAIR + kp
                    nc.sync.dma_start(
                        w8_sb[:, i * 2 * D : (i + 1) * 2 * D],
                        w8_p[i * P : (i + 1) * P, :],
                    )
        if KB:
            wb_sb = const.tile([P, C * KB * D], bf16)      # [:, (c*KB+kb)*D : +D]
            for kb2 in range(0, KB, 2):
                nkb = min(2, KB - kb2)
                for c in range(C):
                    nc.sync.dma_start(
                        wb_sb[:, (c * KB + kb2) * D : (c * KB + kb2 + nkb) * D].rearrange(
                            "p (k d) -> p k d", k=nkb
                        ),
                        wb_p[(c * KB + kb2) * P : (c * KB + kb2 + nkb) * P, :].rearrange(
                            "(k p) d -> p k d", p=P
                        ),
                    )
        ones1 = const.tile([1, P], bf16)
        nc.vector.memset(ones1[:], 1.0)
        pwb_sb = const.tile([P, 1], f32)
        nc.vector.memset(pwb_sb[:], pw_b_f)
        zero_sb = const.tile([P, 1], f32)
        nc.vector.memset(zero_sb[:], 0.0)

        def lhs_feat(k, bt):
            return featT_sb[:, k * BL + bt * P : k * BL + (bt + 1) * P]

        # ---- gate pre-pass: all gate logits in one PSUM bank [P, NBT*9]
        # while the W stream saturates HBM. k-outer so matmuls start as
        # soon as featT[k] lands.
        pred_w_all = const.tile([P, NBT], f32)
        fw_all = const.tile([P, NBT], f32)
        pw8_all = const.tile([P, NBT], f32)
        # One full PSUM bank; a single accumulation group for all 16 b-tiles'
        # 9-col regions: start=True pending-zeroes the WHOLE bank (2KB zero
        # region), each region's first write then overwrites, later writes
        # accumulate. stop only on the very last matmul into the bank.
        pg_all = psum.tile([P, H], f32, tag="pp", name="pg")
        for k in range(KC):
            for bt in range(NBT):
                nc.tensor.matmul(
                    pg_all[:, bt * 9 : (bt + 1) * 9],
                    lhs_feat(k, bt),
                    gmat_sb[:, k * 9 : (k + 1) * 9],
                    start=(k == 0 and bt == 0),
                    stop=False,
                )
        for bt in range(NBT):
            nc.tensor.matmul(
                pg_all[:, bt * 9 : (bt + 1) * 9],
                ones1[:],
                gbeta_sb[:],
                start=False,
                stop=(bt == NBT - 1),
            )
        for bt in range(NBT):
            junk = gate_pool.tile([P, C], f32, name="junk")
            junk2 = gate_pool.tile([P, C], f32, name="junk2")
            sp = gate_pool.tile([P, 1], f32, name="sp")
            # (tensor_tensor_reduce faults the DVE on this HW — use
            # DVE multiply + ACT free-dim accumulator instead)
            nc.vector.tensor_tensor(
                junk[:],
                pg_all[:, bt * 9 : bt * 9 + C],
                prob_all[:, bt * C : (bt + 1) * C],
                op=ALU.mult,
            )
            nc.scalar.activation(junk2[:], junk[:], AF.Copy, accum_out=sp[:])
            nc.scalar.activation(
                pred_w_all[:, bt : bt + 1], sp[:], AF.Sigmoid, bias=pwb_sb[:]
            )
            nc.scalar.activation(
                fw_all[:, bt : bt + 1],
                pg_all[:, bt * 9 + C : bt * 9 + C + 1],
                AF.Sigmoid,
                bias=zero_sb[:],
            )
            if NPAIR:
                nc.vector.tensor_scalar_mul(
                    pw8_all[:, bt : bt + 1],
                    pred_w_all[:, bt : bt + 1],
                    1.0 / W8SCALE,
                )

        # ---- main loop over B-tiles ----
        # PSUM tags: f0/f1 = fp8 banks (single-buffered; evacuated early in
        # the b-tile), b0..b3 = bias+bf16 banks (alternating pairs so the
        # next tile's bias matmul never waits on this tile's evacuation).
        for bt in range(NBT):
            q, qi = divmod(bt, 4)
            if qi == 0:
                # replicated p quads [P, 512] fp32 for 4 b-tiles, built on
                # the otherwise-idle GpSimd engine
                preps = []
                for c in range(C):
                    stg = prep_pool.tile([1, 4 * P], f32, name=f"pstg{c}", tag="ps", bufs=2)
                    nc.gpsimd.dma_start(
                        stg[:], probT32_p[c : c + 1, q * 4 * P : (q + 1) * 4 * P]
                    )
                    pr = prep_pool.tile([P, 4 * P], f32, name=f"prep{c}", tag=f"pr{c}", bufs=1)
                    nc.gpsimd.partition_broadcast(pr[:], stg[:])
                    preps.append(pr)

            def prep_slice(c):
                return preps[c][:, qi * P : (qi + 1) * P]

            # -- prescale: featT_sc[c] = featT * p[b, c] (DVE) --
            sc8s = {}
            for kp in range(NPAIR):
                for c in range(C):
                    t = sc_pool.tile(
                        [P, 2 * P], fp8e4, name=f"s8_{c}_{kp}", tag=f"s8_{c}_{kp}", bufs=2
                    )
                    for ko in range(2):
                        nc.vector.tensor_tensor(
                            t[:, ko * P : (ko + 1) * P],
                            lhs_feat(2 * kp + ko, bt),
                            prep_slice(c),
                            op=ALU.mult,
                        )
                    sc8s[(c, kp)] = t
            scbs = {}
            for kb in range(KB):
                for c in range(C):
                    t = sc_pool.tile(
                        [P, P], bf16, name=f"sb_{c}_{kb}", tag=f"sb_{c}_{kb}", bufs=2
                    )
                    nc.vector.tensor_tensor(
                        t[:], lhs_feat(KOFF + kb, bt), prep_slice(c), op=ALU.mult
                    )
                    scbs[(c, kb)] = t

            # -- fp8 DoubleRow groups (first: f-banks freed early last tile) --
            if NPAIR:
                pf = [psum.tile([P, H], f32, tag=f"f{h}", name=f"pf{h}") for h in range(2)]
                for kp in range(NPAIR):
                    for c in range(C):
                        lhs3 = sc8s[(c, kp)][:].rearrange("p (two m) -> p two m", two=2)
                        i = c * NPAIR + kp
                        mv = w8_sb[:, i * 2 * D : (i + 1) * 2 * D].rearrange(
                            "p (two hh o) -> p two hh o", two=2, hh=2
                        )
                        for h in range(2):
                            nc.tensor.matmul(
                                pf[h][:],
                                lhs3,
                                mv[:, :, h, :],
                                start=(kp == 0 and c == 0),
                                stop=(kp == NPAIR - 1 and c == C - 1),
                                perf_mode=DR,
                            )

            # -- bias + bf16 groups into alternating bank pair --
            bp = bt % 2
            pb = [
                psum.tile([P, H], f32, tag=f"b{2 * bp + h}", name=f"pb{h}")
                for h in range(2)
            ]
            lhs_probT = probT_sb[:, bt * P : (bt + 1) * P]
            nc.tensor.matmul(pb[0][:], lhs_probT, bb_sb[:, 0:H], start=True, stop=False)
            nc.tensor.matmul(pb[1][:], lhs_probT, bb_sb[:, H:D], start=True, stop=False)
            for kb in range(KB):
                for c in range(C):
                    lhs = scbs[(c, kb)]
                    for h in range(2):
                        nc.tensor.matmul(
                            pb[h][:],
                            lhs[:],
                            wb_sb[:, (c * KB + kb) * D + h * H : (c * KB + kb) * D + h * H + H],
                            start=False,
                            stop=(kb == KB - 1 and c == C - 1),
                        )

            # -- evacuation/epilogue --
            t1s = []
            if NPAIR:
                for h in range(2):
                    t1 = stage_pool.tile([P, H], bf16, name=f"t1{h}", tag=f"t1{h}", bufs=1)
                    nc.scalar.activation(
                        t1[:], pf[h][:], AF.Copy, scale=pw8_all[:, bt : bt + 1]
                    )
                    t1s.append(t1)
            feat_sb = feat_pool.tile([P, D], bf16)
            nc.sync.dma_start(feat_sb[:], feat_p[bt * P : (bt + 1) * P, :])
            for h in range(2):
                t0 = stage_pool.tile([P, H], f32, name=f"t0{h}", tag=f"t0{h}", bufs=2)
                nc.scalar.activation(
                    t0[:], pb[h][:], AF.Copy, scale=pred_w_all[:, bt : bt + 1]
                )
                ft = stage_pool.tile([P, H], bf16, name=f"ft{h}", tag=f"ft{h}", bufs=1)
                nc.scalar.activation(
                    ft[:],
                    feat_sb[:, h * H : (h + 1) * H],
                    AF.Copy,
                    scale=fw_all[:, bt : bt + 1],
                )
                if NPAIR:
                    nc.vector.tensor_tensor(t0[:], t0[:], t1s[h][:], op=ALU.add)
                nc.vector.tensor_tensor(t0[:], t0[:], ft[:], op=ALU.add)
                nc.sync.dma_start(
                    out_p[bt * P : (bt + 1) * P, h * H : (h + 1) * H], t0[:]
                )

    if os.environ.get("KERNEL_NO_LDW_DEDUPE") != "1":
        _dedupe_ldweights(nc)
    nc.compile()
    return nc


def _host_prep(feature, prob, W, b, pw_w, pw_b_f, fw_b_f, fw_w):
    """Replicated (non-sharded) host-side weight prep."""
    Wt = np.ascontiguousarray(W.transpose(0, 2, 1))          # [C, d, o]
    host = {}
    if KB:
        host["wb"] = np.ascontiguousarray(Wt[:, KOFF * P :, :]).reshape(
            C * KB * P, D
        ).astype(BF16)
    if NPAIR:
        # rows (c, kp, p), cols (ko, o); value e4m3(128 * W[c, o, (2kp+ko)*P+p])
        w8 = Wt[:, : KOFF * P, :].reshape(C, NPAIR, 2, P, D)
        w8 = np.ascontiguousarray(w8.transpose(0, 1, 3, 2, 4)).reshape(
            C * NPAIR * P, 2 * D
        )
        host["w8"] = (w8 * W8SCALE).astype(FP8)
    G = np.einsum("cod,o->dc", W, pw_w)                      # [D, C]: v_c columns
    G9 = np.concatenate([G, fw_w[:, None]], axis=1)          # [D, 9]
    host["gmat"] = np.ascontiguousarray(
        G9.reshape(KC, P, 9).transpose(1, 0, 2).reshape(P, KC * 9)
    ).astype(BF16)
    host["gbeta"] = np.concatenate([b @ pw_w, [fw_b_f]]).reshape(1, 9).astype(BF16)
    host["bb"] = b.astype(BF16)
    return host


def kernel(feature, prob, W, b, pw_w, pw_b, fw_w, fw_b):
    global LAST_EXEC_NS, LAST_RESULTS
    feature = np.asarray(feature, dtype=np.float32)
    prob = np.asarray(prob, dtype=np.float32)
    W = np.asarray(W, dtype=np.float32)
    b = np.asarray(b, dtype=np.float32)
    pw_w = np.asarray(pw_w, dtype=np.float32)
    fw_w = np.asarray(fw_w, dtype=np.float32)
    pw_b_f = float(np.asarray(pw_b).reshape(-1)[0])
    fw_b_f = float(np.asarray(fw_b).reshape(-1)[0])

    host = _host_prep(feature, prob, W, b, pw_w, pw_b_f, fw_b_f, fw_w)

    in_maps = []
    for i in range(NCORES):
        sl = slice(i * BL, (i + 1) * BL)
        m = {
            "featT": np.ascontiguousarray(feature[sl].T).astype(BF16),
            "feat": feature[sl].astype(BF16),
            "prob": np.ascontiguousarray(prob[sl]),
            "probT": np.ascontiguousarray(prob[sl].T).astype(BF16),
            "probT32": np.ascontiguousarray(prob[sl].T),
        }
        m.update(host)
        in_maps.append(m)

    nc = _build_graph(pw_b_f)
    trace = bool(int(os.environ.get("KERNEL_TRACE", "0")))
    if trace:
        _install_profile_shim()
    res = run_bass_kernel_spmd(
        nc, in_maps, core_ids=list(range(NCORES)), trace=trace
    )
    LAST_EXEC_NS = res.exec_time_ns
    LAST_RESULTS = res
    out = np.concatenate([res.results[i]["out"] for i in range(NCORES)], axis=0)
    return np.asarray(out, dtype=np.float32)
